# revision 4
# baseline (speedup 1.0000x reference)
"""Grouped MLP (MoE expert MLP, ragged token groups) on 8 TRN2 NeuronCores.

Strategy: 8-way tensor-parallel split of the intermediate dim F. Every
core processes ALL tokens with its F/8 = 512 column slice of w1 (and the
matching 512-row slice of w2), producing a partial fc2 sum; the host adds
the 8 fp16 partials and transposes to [T, H].

Why this layout: tokens are grouped contiguously by expert, so each core
walks experts 0..7 in order over the token stream — expert identity,
weight offsets, and chunk widths are all STATIC (no runtime indexing, no
token scheduling / padding). Chunk widths are exact (<= 512, the PSUM
bank limit), so the PE does exactly T*64 cycles of matmul work per core
-- the bf16 roofline for this decomposition. Per-expert weights are tiny
(2 MB/core), so weight prefetch hides trivially under compute.

All DRAM tensors are packed chunk-major, [128, ...] with each chunk's
(or expert's) per-partition data one contiguous run (~7-8 KB): DMA
descriptors are fat, which is what the DMA engines need to run at full
rate (short per-row descriptors measured ~3x slower).

  for e in experts (static):   DMA w1/w2 slices (double-buffered)
    for each chunk of expert e's tokens (static, width w <= 512):
      DMA xT [128, 8ht*w] -> fc1 (4 f-tiles x 8 h-accum matmuls)
      -> Gelu -> fc2 (8 h-tiles x 4 f-accum) -> yT fp16 partial -> DMA

Everything is statically unrolled inside a runtime `reps` loop (timing
only); Tile overlaps all DMA with compute.
"""

import numpy as np
import ml_dtypes

import concourse.bass as bass  # noqa: F401  (kept for parity with tooling)
import concourse.mybir as mybir
import concourse.tile as tile
from concourse import bacc
from concourse.bass_utils import run_bass_kernel_spmd

# Problem shape (fixed by the task).
T, H, F, E = 16384, 1024, 4096, 8
NCORES = 8
FS = F // NCORES      # 512: per-core F slice
HT = H // 128         # 8 h-tiles
FT = FS // 128        # 4 f-tiles per core
WMAX = 512            # max matmul moving width (PSUM bank = 512 f32)

_BF16 = mybir.dt.bfloat16
_F16 = mybir.dt.float16
_F32 = mybir.dt.float32
_I32 = mybir.dt.int32

GELU_FUNC = mybir.ActivationFunctionType.Gelu

_cache = {}


def _chunks(counts):
    """counts[E] -> list of (expert, col_start, width) with width <= WMAX.

    Each expert's contiguous token run is split into near-equal chunks, so
    there is no padding at all: sum of widths == sum(counts)."""
    chunks = []
    col = 0
    for e in range(E):
        c = int(counts[e])
        if c <= 0:
            continue
        k = -(-c // WMAX)
        base, rem = divmod(c, k)
        off = 0
        for i in range(k):
            w = base + (1 if i < rem else 0)
            chunks.append((e, col + off, w))
            off += w
        col += c
    return chunks


def _build(counts_key):
    if counts_key in _cache:
        return _cache[counts_key]
    chunks = _chunks(counts_key)

    nc = bacc.Bacc("TRN2", target_bir_lowering=False, debug=False,
                   num_devices=NCORES)
    xt_d = nc.declare_dram_parameter("xt", [128, HT * T], _BF16,
                                     isOutput=False)
    w1_d = nc.declare_dram_parameter("w1", [128, E * HT * FS], _BF16,
                                     isOutput=False)
    w2_d = nc.declare_dram_parameter("w2", [128, E * FT * H], _BF16,
                                     isOutput=False)
    meta_d = nc.declare_dram_parameter("meta", [1, 1], _I32, isOutput=False)
    yt_d = nc.declare_dram_parameter("yt", [128, HT * T], _F16,
                                     isOutput=True)

    with tile.TileContext(nc) as tc:
        with (
            tc.tile_pool(name="meta", bufs=1) as mpool,
            tc.tile_pool(name="w1", bufs=3) as w1pool,
            tc.tile_pool(name="w2", bufs=3) as w2pool,
            tc.tile_pool(name="x", bufs=4) as xpool,
            tc.tile_pool(name="act", bufs=2) as apool,
            tc.tile_pool(name="y", bufs=4) as ypool,
            tc.tile_pool(name="ps1", bufs=4, space="PSUM") as ps1pool,
            tc.tile_pool(name="ps2", bufs=4, space="PSUM") as ps2pool,
        ):
            mt = mpool.tile([1, 1], _I32)
            nc.sync.dma_start(mt[:], meta_d[:])
            # skip_runtime_bounds_check: runtime assert traps kill the
            # axon/PJRT execution path.
            reps = nc.values_load(mt[:1, 0:1], min_val=1, max_val=100000,
                                  skip_runtime_bounds_check=True)

            rep_loop = tc.For_i(0, reps, name="reps",
                                staggered_reset=True,
                                hint_engines=mybir.ALL_ENGINES)
            rep_loop.__enter__()
            cur_e = None
            w1sb = w2sb = None
            for (e, col, w) in chunks:
                if e != cur_e:
                    cur_e = e
                    w1sb = w1pool.tile([128, HT * FS], _BF16, tag="w1sb")
                    w2sb = w2pool.tile([128, FT * H], _BF16, tag="w2sb")
                    # Split weight loads (parallel DMA + lets the first
                    # h/f tiles start before the whole load lands).
                    wq = HT * FS // 4
                    for q in range(4):
                        nc.sync.dma_start(
                            w1sb[:, q * wq:(q + 1) * wq],
                            w1_d[:, e * HT * FS + q * wq:
                                 e * HT * FS + (q + 1) * wq])
                    for q in range(4):
                        nc.sync.dma_start(
                            w2sb[:, q * wq:(q + 1) * wq],
                            w2_d[:, e * FT * H + q * wq:
                                 e * FT * H + (q + 1) * wq])
                o = HT * col
                xt_sb = xpool.tile([128, HT * w], _BF16, tag="xt")
                nc.sync.dma_start(xt_sb[:, :4 * w], xt_d[:, o:o + 4 * w])
                nc.sync.dma_start(xt_sb[:, 4 * w:], xt_d[:, o + 4 * w:
                                                         o + 8 * w])
                act_sb = apool.tile([128, FT * w], _BF16, tag="act")
                for f in range(FT):
                    ps = ps1pool.tile([128, WMAX], _F32, tag="ps1")
                    for h in range(HT):
                        nc.tensor.matmul(
                            ps[:, :w],
                            w1sb[:, h * FS + f * 128:h * FS + (f + 1) * 128],
                            xt_sb[:, h * w:(h + 1) * w],
                            start=(h == 0), stop=(h == HT - 1))
                    nc.scalar.activation(act_sb[:, f * w:(f + 1) * w],
                                         ps[:, :w], GELU_FUNC)
                yt_sb = ypool.tile([128, HT * w], _F16, tag="yt")
                for h in range(HT):
                    ps2 = ps2pool.tile([128, WMAX], _F32, tag="ps2")
                    for f in range(FT):
                        nc.tensor.matmul(
                            ps2[:, :w],
                            w2sb[:, f * H + h * 128:f * H + (h + 1) * 128],
                            act_sb[:, f * w:(f + 1) * w],
                            start=(f == 0), stop=(f == FT - 1))
                    nc.vector.tensor_copy(yt_sb[:, h * w:(h + 1) * w],
                                          ps2[:, :w])
                nc.sync.dma_start(yt_d[:, o:o + 4 * w], yt_sb[:, :4 * w])
                nc.sync.dma_start(yt_d[:, o + 4 * w:o + 8 * w],
                                  yt_sb[:, 4 * w:])
            rep_loop.__exit__(None, None, None)
    nc.compile()
    _cache[counts_key] = nc
    return nc


def _pack_rows(mat, nt):
    """[nt*128, cols] -> [128, nt*cols] with per-partition [nt, cols]
    contiguous blocks."""
    cols = mat.shape[1]
    return np.ascontiguousarray(
        mat.reshape(nt, 128, cols).transpose(1, 0, 2).reshape(128, nt * cols))


def _make_inputs(x, w1, w2, reps=1):
    xb = x.astype(ml_dtypes.bfloat16)
    w1b = w1.astype(ml_dtypes.bfloat16)
    w2b = w2.astype(ml_dtypes.bfloat16)
    # x: [T, H] -> packed [128, HT*T], token-major per chunk == global
    # token-major: block for token t is [HT] x col t -> pack whole thing
    # as [128, ht, t] contiguous in (ht, t)? No: chunk-major == contiguous
    # token ranges, and within a range [ht][tok] blocks. Global layout
    # [128, ht, T] would interleave ht with FULL T; we need per-chunk
    # blocks, i.e. [128, sum_j (ht * w_j)]. Since chunks tile the token
    # axis in order, pack per chunk below in _pack_x.
    meta = np.full((1, 1), reps, np.int32)
    in_maps = []
    for c in range(NCORES):
        w1c = np.concatenate(
            [_pack_rows(w1b[e, :, c * FS:(c + 1) * FS], HT)
             for e in range(E)], axis=1)
        w2c = np.concatenate(
            [_pack_rows(w2b[e, c * FS:(c + 1) * FS, :], FT)
             for e in range(E)], axis=1)
        in_maps.append({"w1": w1c, "w2": w2c, "meta": meta})
    return xb, in_maps


def _pack_x(xb, chunks):
    xtT = np.ascontiguousarray(xb.T)  # [H, T]
    parts = []
    for (e, col, w) in chunks:
        parts.append(_pack_rows(xtT[:, col:col + w], HT))
    return np.concatenate(parts, axis=1)  # [128, HT*T]


def _gather(results, chunks):
    ys = np.zeros((128, HT * T), np.float32)
    for r in results:
        ys += np.asarray(r["yt"], np.float32)
    out = np.empty((T, H), np.float32)
    for (e, col, w) in chunks:
        o = HT * col
        blk = ys[:, o:o + HT * w].reshape(128, HT, w)
        out[col:col + w] = blk.transpose(2, 1, 0).reshape(w, H)
    return out


def prepare(x, w1, w2, counts):
    """For test harness: compiled program + in_maps factory with a reps knob."""
    key = tuple(int(c) for c in counts)
    nc = _build(key)
    chunks = _chunks(key)

    def make_in_maps(reps):
        xb, in_maps = _make_inputs(x, w1, w2, reps=reps)
        xt = _pack_x(xb, chunks)
        for m in in_maps:
            m["xt"] = xt
        return in_maps

    return nc, make_in_maps


def kernel(permuted_local_hidden_states, weight1, weight2, tokens_per_expert):
    x = np.asarray(permuted_local_hidden_states, np.float32)
    w1 = np.asarray(weight1, np.float32)
    w2 = np.asarray(weight2, np.float32)
    counts = np.asarray(tokens_per_expert).astype(np.int64)
    assert int(counts.sum()) == T, counts

    key = tuple(int(c) for c in counts)
    nc = _build(key)
    chunks = _chunks(key)
    xb, in_maps = _make_inputs(x, w1, w2)
    xt = _pack_x(xb, chunks)
    for m in in_maps:
        m["xt"] = xt
    res = run_bass_kernel_spmd(nc, in_maps, list(range(NCORES)))
    return _gather(res.results, chunks)


# revision 5
# speedup vs baseline: 1.0041x; 1.0041x over previous
"""Grouped MLP (MoE expert MLP, ragged token groups) on 8 TRN2 NeuronCores.

Strategy: 8-way tensor-parallel split of the intermediate dim F. Every
core processes ALL tokens with its F/8 = 512 column slice of w1 (and the
matching 512-row slice of w2), producing a partial fc2 sum; the host adds
the 8 fp16 partials and transposes to [T, H].

Why this layout: tokens are grouped contiguously by expert, so each core
walks experts 0..7 in order over the token stream — expert identity,
weight offsets, and chunk widths are all STATIC (no runtime indexing, no
token scheduling / padding). Chunk widths are exact (<= 512, the PSUM
bank limit), so the PE does exactly T*64 cycles of matmul work per core
-- the bf16 roofline for this decomposition. Per-expert weights are tiny
(2 MB/core), so weight prefetch hides trivially under compute.

All DRAM tensors are packed chunk-major, [128, ...] with each chunk's
(or expert's) per-partition data one contiguous run (~7-8 KB): DMA
descriptors are fat, which is what the DMA engines need to run at full
rate (short per-row descriptors measured ~3x slower).

  for e in experts (static):   DMA w1/w2 slices (double-buffered)
    for each chunk of expert e's tokens (static, width w <= 512):
      DMA xT [128, 8ht*w] -> fc1 (4 f-tiles x 8 h-accum matmuls)
      -> Gelu -> fc2 (8 h-tiles x 4 f-accum) -> yT fp16 partial -> DMA

Everything is statically unrolled inside a runtime `reps` loop (timing
only); Tile overlaps all DMA with compute.
"""

import numpy as np
import ml_dtypes

import concourse.bass as bass  # noqa: F401  (kept for parity with tooling)
import concourse.mybir as mybir
import concourse.tile as tile
from concourse import bacc
from concourse.bass_utils import run_bass_kernel_spmd

# Problem shape (fixed by the task).
T, H, F, E = 16384, 1024, 4096, 8
NCORES = 8
FS = F // NCORES      # 512: per-core F slice
HT = H // 128         # 8 h-tiles
FT = FS // 128        # 4 f-tiles per core
WMAX = 512            # max matmul moving width (PSUM bank = 512 f32)

_BF16 = mybir.dt.bfloat16
_F16 = mybir.dt.float16
_F32 = mybir.dt.float32
_I32 = mybir.dt.int32

GELU_FUNC = mybir.ActivationFunctionType.Gelu

_cache = {}


def _chunks(counts):
    """counts[E] -> list of (expert, col_start, width) with width <= WMAX.

    Each expert's contiguous token run is split into near-equal chunks, so
    there is no padding at all: sum of widths == sum(counts)."""
    chunks = []
    col = 0
    for e in range(E):
        c = int(counts[e])
        if c <= 0:
            continue
        k = -(-c // WMAX)
        base, rem = divmod(c, k)
        off = 0
        for i in range(k):
            w = base + (1 if i < rem else 0)
            chunks.append((e, col + off, w))
            off += w
        col += c
    return chunks


def _build(counts_key):
    if counts_key in _cache:
        return _cache[counts_key]
    chunks = _chunks(counts_key)

    nc = bacc.Bacc("TRN2", target_bir_lowering=False, debug=False,
                   num_devices=NCORES)
    xt_d = nc.declare_dram_parameter("xt", [128, HT * T], _BF16,
                                     isOutput=False)
    w1_d = nc.declare_dram_parameter("w1", [128, E * HT * FS], _BF16,
                                     isOutput=False)
    w2_d = nc.declare_dram_parameter("w2", [128, E * FT * H], _BF16,
                                     isOutput=False)
    meta_d = nc.declare_dram_parameter("meta", [1, 1], _I32, isOutput=False)
    yt_d = nc.declare_dram_parameter("yt", [128, HT * T], _F16,
                                     isOutput=True)

    with tile.TileContext(nc) as tc:
        with (
            tc.tile_pool(name="meta", bufs=1) as mpool,
            tc.tile_pool(name="w1", bufs=3) as w1pool,
            tc.tile_pool(name="w2", bufs=3) as w2pool,
            tc.tile_pool(name="x", bufs=4) as xpool,
            tc.tile_pool(name="act", bufs=2) as apool,
            tc.tile_pool(name="y", bufs=4) as ypool,
            tc.tile_pool(name="ps1", bufs=4, space="PSUM") as ps1pool,
            tc.tile_pool(name="ps2", bufs=4, space="PSUM") as ps2pool,
        ):
            mt = mpool.tile([1, 1], _I32)
            nc.sync.dma_start(mt[:], meta_d[:])
            # skip_runtime_bounds_check: runtime assert traps kill the
            # axon/PJRT execution path.
            reps = nc.values_load(mt[:1, 0:1], min_val=1, max_val=100000,
                                  skip_runtime_bounds_check=True)

            rep_loop = tc.For_i(0, reps, name="reps",
                                staggered_reset=True,
                                hint_engines=mybir.ALL_ENGINES)
            rep_loop.__enter__()
            cur_e = None
            w1sb = w2sb = None
            for (e, col, w) in chunks:
                if e != cur_e:
                    cur_e = e
                    w1sb = w1pool.tile([128, HT * FS], _BF16, tag="w1sb")
                    w2sb = w2pool.tile([128, FT * H], _BF16, tag="w2sb")
                    # Split weight loads (parallel DMA + lets the first
                    # h/f tiles start before the whole load lands).
                    wq = HT * FS // 4
                    for q in range(4):
                        nc.scalar.dma_start(
                            w1sb[:, q * wq:(q + 1) * wq],
                            w1_d[:, e * HT * FS + q * wq:
                                 e * HT * FS + (q + 1) * wq])
                    for q in range(4):
                        nc.scalar.dma_start(
                            w2sb[:, q * wq:(q + 1) * wq],
                            w2_d[:, e * FT * H + q * wq:
                                 e * FT * H + (q + 1) * wq])
                o = HT * col
                xt_sb = xpool.tile([128, HT * w], _BF16, tag="xt")
                nc.scalar.dma_start(xt_sb[:, :4 * w], xt_d[:, o:o + 4 * w])
                nc.scalar.dma_start(xt_sb[:, 4 * w:], xt_d[:, o + 4 * w:
                                                           o + 8 * w])
                act_sb = apool.tile([128, FT * w], _BF16, tag="act")
                for f in range(FT):
                    ps = ps1pool.tile([128, WMAX], _F32, tag="ps1")
                    for h in range(HT):
                        nc.tensor.matmul(
                            ps[:, :w],
                            w1sb[:, h * FS + f * 128:h * FS + (f + 1) * 128],
                            xt_sb[:, h * w:(h + 1) * w],
                            start=(h == 0), stop=(h == HT - 1))
                    nc.scalar.activation(act_sb[:, f * w:(f + 1) * w],
                                         ps[:, :w], GELU_FUNC)
                yt_sb = ypool.tile([128, HT * w], _F16, tag="yt")
                for h in range(HT):
                    ps2 = ps2pool.tile([128, WMAX], _F32, tag="ps2")
                    for f in range(FT):
                        nc.tensor.matmul(
                            ps2[:, :w],
                            w2sb[:, f * H + h * 128:f * H + (h + 1) * 128],
                            act_sb[:, f * w:(f + 1) * w],
                            start=(f == 0), stop=(f == FT - 1))
                    nc.vector.tensor_copy(yt_sb[:, h * w:(h + 1) * w],
                                          ps2[:, :w])
                nc.sync.dma_start(yt_d[:, o:o + 4 * w], yt_sb[:, :4 * w])
                nc.sync.dma_start(yt_d[:, o + 4 * w:o + 8 * w],
                                  yt_sb[:, 4 * w:])
            rep_loop.__exit__(None, None, None)
    nc.compile()
    _cache[counts_key] = nc
    return nc


def _pack_rows(mat, nt):
    """[nt*128, cols] -> [128, nt*cols] with per-partition [nt, cols]
    contiguous blocks."""
    cols = mat.shape[1]
    return np.ascontiguousarray(
        mat.reshape(nt, 128, cols).transpose(1, 0, 2).reshape(128, nt * cols))


def _make_inputs(x, w1, w2, reps=1):
    xb = x.astype(ml_dtypes.bfloat16)
    w1b = w1.astype(ml_dtypes.bfloat16)
    w2b = w2.astype(ml_dtypes.bfloat16)
    # x: [T, H] -> packed [128, HT*T], token-major per chunk == global
    # token-major: block for token t is [HT] x col t -> pack whole thing
    # as [128, ht, t] contiguous in (ht, t)? No: chunk-major == contiguous
    # token ranges, and within a range [ht][tok] blocks. Global layout
    # [128, ht, T] would interleave ht with FULL T; we need per-chunk
    # blocks, i.e. [128, sum_j (ht * w_j)]. Since chunks tile the token
    # axis in order, pack per chunk below in _pack_x.
    meta = np.full((1, 1), reps, np.int32)
    in_maps = []
    for c in range(NCORES):
        w1c = np.concatenate(
            [_pack_rows(w1b[e, :, c * FS:(c + 1) * FS], HT)
             for e in range(E)], axis=1)
        w2c = np.concatenate(
            [_pack_rows(w2b[e, c * FS:(c + 1) * FS, :], FT)
             for e in range(E)], axis=1)
        in_maps.append({"w1": w1c, "w2": w2c, "meta": meta})
    return xb, in_maps


def _pack_x(xb, chunks):
    xtT = np.ascontiguousarray(xb.T)  # [H, T]
    parts = []
    for (e, col, w) in chunks:
        parts.append(_pack_rows(xtT[:, col:col + w], HT))
    return np.concatenate(parts, axis=1)  # [128, HT*T]


def _gather(results, chunks):
    ys = np.zeros((128, HT * T), np.float32)
    for r in results:
        ys += np.asarray(r["yt"], np.float32)
    out = np.empty((T, H), np.float32)
    for (e, col, w) in chunks:
        o = HT * col
        blk = ys[:, o:o + HT * w].reshape(128, HT, w)
        out[col:col + w] = blk.transpose(2, 1, 0).reshape(w, H)
    return out


def prepare(x, w1, w2, counts):
    """For test harness: compiled program + in_maps factory with a reps knob."""
    key = tuple(int(c) for c in counts)
    nc = _build(key)
    chunks = _chunks(key)

    def make_in_maps(reps):
        xb, in_maps = _make_inputs(x, w1, w2, reps=reps)
        xt = _pack_x(xb, chunks)
        for m in in_maps:
            m["xt"] = xt
        return in_maps

    return nc, make_in_maps


def kernel(permuted_local_hidden_states, weight1, weight2, tokens_per_expert):
    x = np.asarray(permuted_local_hidden_states, np.float32)
    w1 = np.asarray(weight1, np.float32)
    w2 = np.asarray(weight2, np.float32)
    counts = np.asarray(tokens_per_expert).astype(np.int64)
    assert int(counts.sum()) == T, counts

    key = tuple(int(c) for c in counts)
    nc = _build(key)
    chunks = _chunks(key)
    xb, in_maps = _make_inputs(x, w1, w2)
    xt = _pack_x(xb, chunks)
    for m in in_maps:
        m["xt"] = xt
    res = run_bass_kernel_spmd(nc, in_maps, list(range(NCORES)))
    return _gather(res.results, chunks)
